# revision 33
# baseline (speedup 1.0000x reference)
"""Associative-embedding loss kernel for 8 Trainium2 NeuronCores.

Math: per image b, with tl[n,c] = pred[b,c,ty,tx] and br[n,c] = target[b,c,by,bx]
gathered at the N=128 match points:
  pull_b = sum_{n,c} (tl-br)^2 / (2N)
  s'[n]  = sum_c (tl+br),  A'[i,j] = s'[i]-s'[j]   (A = A'/2)
  push_b = (0.5*(sum|A'+2| - sum|A'|) - N) / (N(N-1))
using sum_{ij} relu(1-|A|) = sum|A+1| - sum|A| for antisymmetric A.

Strategy: data-parallel over B (8 images per core). The host shards each
core's 128x2 match points into three small uploads (~320KB/core); HW
indirect DMA is limited to one index per partition per instruction
(~1.3us each, 16 per core), which made an on-device gather the dominant
cost, so the point extraction happens host-side and every loss FLOP runs
on device.

The corner/channel sums that produce s' are folded into the pairwise
matmul contraction (K=128, bf16):
  lhsT rows 8b+q       = raw values v[b, i, q]  (q = 8 corner x channel)
  lhsT rows 64+..      = -1
  rhs rows 8b+q        = 1 on column block b (constant indicator)
  rhs rows 64+8b+q     = v[b, j, q] on column block b, zeros elsewhere
  => out[i, 128b+j] = sum_q v[b,i,q] - sum_q v[b,j,q] = s'_b[i] - s'_b[j]
for all 8 images into one two-bank PSUM tile [128, 1024]. The Scalar
engine accumulates |A'+2| in one pass (Abs with bias via accum_out), the
Vector engine row-reduces |A'| in one pass, and pull comes from an fp32
subtract (GpSimd, 32 partitions) + square-accumulate (DVE) on a separate
[64, 128] upload. rh is split by column halves across two DMA queues so
the first matmul starts half a transfer earlier. bf16 rounding only
perturbs s' by ~0.4%, far inside the 2e-2 gate; pull stays fp32 exact.
Each core returns [128, 8] partial sums folded on the host in fp64.
"""

import numpy as np

B, C, H, W, N = 64, 4, 256, 256, 128
M = 8            # cores
BL = B // M      # images per core
Q = 2 * C        # corner x channel values per point

_GRAPH = None

# constant indicator rows: row 8b+q is 1 on column block b
_IND = np.repeat(np.kron(np.eye(8), np.ones((1, N))), Q, axis=0)


def _build_graph():
    import concourse.bacc as bacc
    import concourse.mybir as mybir
    from concourse.tile import TileContext

    f32 = mybir.dt.float32
    bf16 = mybir.dt.bfloat16
    Alu = mybir.AluOpType
    Act = mybir.ActivationFunctionType
    Axis = mybir.AxisListType

    nc = bacc.Bacc()
    lt_d = nc.declare_dram_parameter("lt", [128, 128], bf16, isOutput=False)
    rh_d = nc.declare_dram_parameter("rh", [128, 8 * N], bf16, isOutput=False)
    g_d = nc.declare_dram_parameter("g", [32, 2 * N], f32, isOutput=False)
    ov_d = nc.declare_dram_parameter("ov", [128, 3], f32, isOutput=True)
    os_d = nc.declare_dram_parameter("os", [128, 1], f32, isOutput=True)

    with TileContext(nc) as tc:
        with (
            tc.tile_pool(name="sb", bufs=1) as pool,
            tc.tile_pool(name="ps", bufs=2, space="PSUM") as psum,
        ):
            # rh column halves both on the sync queue (each matmul's gate
            # arrives half a transfer earlier); lt first on the scalar queue
            # so LDWEIGHTS is never the straggler, then g behind it
            rht = pool.tile([128, 8 * N], bf16)
            nc.sync.dma_start(out=rht[:, 0:512], in_=rh_d[:, 0:512])
            nc.sync.dma_start(out=rht[:, 512:1024], in_=rh_d[:, 512:1024])
            ltt = pool.tile([128, 128], bf16)
            nc.scalar.dma_start(out=ltt[:], in_=lt_d[:])
            g = pool.tile([32, 2 * N], f32)
            nc.scalar.dma_start(out=g[:], in_=g_d[:])

            two = pool.tile([128, 1], f32)
            nc.vector.memset(two[:], 2.0)

            # separate result tiles per engine: a shared tile would make the
            # Tile tracker serialize the independent reducers
            accv = pool.tile([128, 3], f32)   # col0 pull (rows 0-31), 1/2 |A'|
            nc.vector.memset(accv[:], 0.0)
            accs = pool.tile([128, 1], f32)   # |A'+2|

            # pull: d = tl - br on GpSimd, square+accumulate on DVE
            dt_ = pool.tile([32, N], f32)
            nc.gpsimd.tensor_sub(dt_[:], g[:, 0:N], g[:, N:2 * N])
            d2 = pool.tile([32, N], f32)
            nc.vector.scalar_tensor_tensor(
                out=d2[:], in0=dt_[:], scalar=0.0, in1=dt_[:],
                op0=Alu.bypass, op1=Alu.mult, accum_out=accv[0:32, 0:1])

            # A'[i, 128b+j] = s'_b[i] - s'_b[j], all 8 images in one
            # two-bank PSUM tile
            bank = psum.tile([128, 8 * N], f32, name="bank", tag="a")
            nc.tensor.matmul(out=bank[:, 0:512], lhsT=ltt[:],
                             rhs=rht[:, 0:512], start=True, stop=True)
            nc.tensor.matmul(out=bank[:, 512:1024], lhsT=ltt[:],
                             rhs=rht[:, 512:1024], start=True, stop=True)

            # accs = rowsum |A'+2| (one pass); accv col1/2 = rowsum |A'|
            # per bank so the first reduce starts right after matmul 1
            scr = pool.tile([128, 8 * N], f32)
            nc.scalar.activation(
                out=scr[:], in_=bank[:], func=Act.Abs, bias=two[:, 0:1],
                scale=1.0, accum_out=accs[:, 0:1])
            nc.vector.tensor_reduce(
                out=accv[:, 1:2], in_=bank[:, 0:512], axis=Axis.X,
                op=Alu.add, apply_absolute_value=True)
            nc.vector.tensor_reduce(
                out=accv[:, 2:3], in_=bank[:, 512:1024], axis=Axis.X,
                op=Alu.add, apply_absolute_value=True)

            nc.sync.dma_start(out=ov_d[:], in_=accv[:])
            nc.scalar.dma_start(out=os_d[:], in_=accs[:])
    nc.finalize()
    return nc


def _get_graph():
    global _GRAPH
    if _GRAPH is None:
        _GRAPH = _build_graph()
    return _GRAPH


def _make_in_maps(pred, target, match):
    import ml_dtypes

    bf16 = ml_dtypes.bfloat16
    barr = np.arange(B)[:, None]
    tl = pred[barr, :, match[:, :, 0, 0], match[:, :, 0, 1]]    # [B, N, C]
    br = target[barr, :, match[:, :, 1, 0], match[:, :, 1, 1]]  # [B, N, C]
    raw = np.concatenate([tl, br], axis=-1)                     # [B, N, Q]
    raw16 = raw.astype(bf16)

    in_maps = []
    for i in range(M):
        sl = slice(i * BL, (i + 1) * BL)
        rc = raw16[sl]                                          # [BL, N, Q]
        lt = np.empty((128, 128), bf16)
        lt[0:64] = rc.transpose(0, 2, 1).reshape(64, N)         # rows 8b+q
        lt[64:128] = bf16(-1.0)
        rh = np.zeros((128, 8 * N), bf16)
        rh[0:64] = _IND
        for b in range(BL):
            rh[64 + Q * b:64 + Q * (b + 1), N * b:N * (b + 1)] = \
                rc[b].transpose(1, 0)
        # g row 4b+c = [tl[b, :, c] | br[b, :, c]]
        g = np.empty((32, 2 * N), np.float32)
        g[:, 0:N] = tl[sl].transpose(0, 2, 1).reshape(32, N)
        g[:, N:2 * N] = br[sl].transpose(0, 2, 1).reshape(32, N)
        in_maps.append({"lt": lt, "rh": rh, "g": g})
    return in_maps


def _finish(core_outs):
    pull_total = 0.0
    m_total = 0.0
    for ov, os_ in core_outs:
        ov = np.asarray(ov, dtype=np.float64)
        os_ = np.asarray(os_, dtype=np.float64)
        pull_total += ov[0:32, 0].sum()
        m_total += (os_[:, 0] - ov[:, 1] - ov[:, 2]).sum()
    # per image: 0.5*(sum|A'+2| - sum|A'|) = P_b + N
    pull_all = 0.25 * pull_total / (2 * N)
    push_all = 0.25 * (0.5 * m_total - B * N) / (N * (N - 1))
    return (np.float32(pull_all), np.float32(push_all))


def kernel(pred, target, match):
    from concourse.bass_utils import run_bass_kernel_spmd

    nc = _get_graph()
    in_maps = _make_in_maps(np.asarray(pred), np.asarray(target), np.asarray(match))
    res = run_bass_kernel_spmd(nc, in_maps, core_ids=list(range(M)))
    return _finish([(r["ov"], r["os"]) for r in res.results])


# revision 38
# speedup vs baseline: 1.2011x; 1.2011x over previous
"""Associative-embedding loss kernel for 8 Trainium2 NeuronCores.

Math: per image b, with tl[n,c] = pred[b,c,ty,tx] and br[n,c] = target[b,c,by,bx]
gathered at the N=128 match points:
  pull_b = sum_{n,c} (tl-br)^2 / (2N)
  s'[n]  = sum_c (tl+br),  A'[i,j] = s'[i]-s'[j]   (A = A'/2)
  push_b = (0.5*(sum|A'+2| - sum|A'|) - N) / (N(N-1))
using sum_{ij} relu(1-|A|) = sum|A+1| - sum|A| for antisymmetric A.

Strategy: data-parallel over B (8 images per core). The host shards each
core's 128x2 match points into three small uploads (~320KB/core); HW
indirect DMA is limited to one index per partition per instruction
(~1.3us each, 16 per core), which made an on-device gather the dominant
cost, so the point extraction happens host-side and every loss FLOP runs
on device.

The corner/channel sums that produce s' are folded into the pairwise
matmul contraction (K=128, bf16):
  lhsT rows 8b+q       = raw values v[b, i, q]  (q = 8 corner x channel)
  lhsT rows 64+..      = -1
  rhs rows 8b+q        = 1 on column block b (constant indicator)
  rhs rows 64+8b+q     = v[b, j, q] on column block b, zeros elsewhere
  => out[i, 128b+j] = sum_q v[b,i,q] - sum_q v[b,j,q] = s'_b[i] - s'_b[j]
for all 8 images into one two-bank PSUM tile [128, 1024]. The Scalar
engine accumulates |A'+2| in one pass (Abs with bias via accum_out), the
Vector engine row-reduces |A'| in one pass, and pull comes from an fp32
subtract (GpSimd, 32 partitions) + square-accumulate (DVE) on a separate
[64, 128] upload. rh is split by column halves across two DMA queues so
the first matmul starts half a transfer earlier. bf16 rounding only
perturbs s' by ~0.4%, far inside the 2e-2 gate; pull stays fp32 exact.
Each core returns [128, 8] partial sums folded on the host in fp64.
"""

import numpy as np

B, C, H, W, N = 64, 4, 256, 256, 128
M = 8            # cores
BL = B // M      # images per core
Q = 2 * C        # corner x channel values per point

_GRAPH = None

# constant indicator rows: row 8b+q is 1 on column block b
_IND = np.repeat(np.kron(np.eye(8), np.ones((1, N))), Q, axis=0)


def _build_graph():
    import concourse.bacc as bacc
    import concourse.mybir as mybir
    from concourse.tile import TileContext

    f32 = mybir.dt.float32
    bf16 = mybir.dt.bfloat16
    Alu = mybir.AluOpType
    Act = mybir.ActivationFunctionType
    Axis = mybir.AxisListType

    nc = bacc.Bacc()
    lt_d = nc.declare_dram_parameter("lt", [128, 128], bf16, isOutput=False)
    rh_d = nc.declare_dram_parameter("rh", [128, 8 * N], bf16, isOutput=False)
    g_d = nc.declare_dram_parameter("g", [32, 2 * N], f32, isOutput=False)
    ov_d = nc.declare_dram_parameter("ov", [128, 3], f32, isOutput=True)
    os_d = nc.declare_dram_parameter("os", [128, 2], f32, isOutput=True)

    with TileContext(nc) as tc:
        with (
            tc.tile_pool(name="sb", bufs=1) as pool,
            tc.tile_pool(name="ps", bufs=2, space="PSUM") as psum,
        ):
            # rh (the matmul gate, largest) as one contiguous transfer on the
            # sync queue; lt first on the scalar queue so LDWEIGHTS is never
            # the straggler, then g behind it
            rht = pool.tile([128, 8 * N], bf16)
            nc.sync.dma_start(out=rht[:], in_=rh_d[:])
            ltt = pool.tile([128, 128], bf16)
            nc.scalar.dma_start(out=ltt[:], in_=lt_d[:])
            g = pool.tile([32, 2 * N], f32)
            nc.scalar.dma_start(out=g[:], in_=g_d[:])

            two = pool.tile([128, 1], f32)
            nc.vector.memset(two[:], 2.0)

            # separate result tiles per engine: a shared tile would make the
            # Tile tracker serialize the independent reducers
            accv = pool.tile([128, 3], f32)   # col0 pull (rows 0-31), 1/2 |A'|
            nc.vector.memset(accv[:], 0.0)
            accs = pool.tile([128, 2], f32)   # |A'+2| per bank

            # pull: d = tl - br on GpSimd, square+accumulate on DVE
            dt_ = pool.tile([32, N], f32)
            nc.gpsimd.tensor_sub(dt_[:], g[:, 0:N], g[:, N:2 * N])
            d2 = pool.tile([32, N], f32)
            nc.vector.scalar_tensor_tensor(
                out=d2[:], in0=dt_[:], scalar=0.0, in1=dt_[:],
                op0=Alu.bypass, op1=Alu.mult, accum_out=accv[0:32, 0:1])

            # A'[i, 128b+j] = s'_b[i] - s'_b[j], 4 images per PSUM bank;
            # separate bank tiles so each bank's reducers start right after
            # its matmul instead of after both
            Pm = mybir.MatmulPerfMode.DoublePixel
            bankA = psum.tile([128, 512], f32, name="bankA", tag="a")
            bankB = psum.tile([128, 512], f32, name="bankB", tag="b")
            nc.tensor.matmul(out=bankA[:], lhsT=ltt[:], rhs=rht[:, 0:512],
                             start=True, stop=True, perf_mode=Pm)
            nc.tensor.matmul(out=bankB[:], lhsT=ltt[:], rhs=rht[:, 512:1024],
                             start=True, stop=True, perf_mode=Pm)

            # accs = rowsum |A'+2| per bank; accv col1/2 = rowsum |A'|
            scr = pool.tile([128, 8 * N], f32)
            nc.scalar.activation(
                out=scr[:, 0:512], in_=bankA[:], func=Act.Abs,
                bias=two[:, 0:1], scale=1.0, accum_out=accs[:, 0:1])
            nc.scalar.activation(
                out=scr[:, 512:1024], in_=bankB[:], func=Act.Abs,
                bias=two[:, 0:1], scale=1.0, accum_out=accs[:, 1:2])
            nc.vector.tensor_reduce(
                out=accv[:, 1:2], in_=bankA[:], axis=Axis.X,
                op=Alu.add, apply_absolute_value=True)
            nc.vector.tensor_reduce(
                out=accv[:, 2:3], in_=bankB[:], axis=Axis.X,
                op=Alu.add, apply_absolute_value=True)

            nc.sync.dma_start(out=ov_d[:], in_=accv[:])
            nc.scalar.dma_start(out=os_d[:], in_=accs[:])
    nc.finalize()
    return nc


def _get_graph():
    global _GRAPH
    if _GRAPH is None:
        _GRAPH = _build_graph()
    return _GRAPH


def _make_in_maps(pred, target, match):
    import ml_dtypes

    bf16 = ml_dtypes.bfloat16
    barr = np.arange(B)[:, None]
    tl = pred[barr, :, match[:, :, 0, 0], match[:, :, 0, 1]]    # [B, N, C]
    br = target[barr, :, match[:, :, 1, 0], match[:, :, 1, 1]]  # [B, N, C]
    raw = np.concatenate([tl, br], axis=-1)                     # [B, N, Q]
    raw16 = raw.astype(bf16)

    in_maps = []
    for i in range(M):
        sl = slice(i * BL, (i + 1) * BL)
        rc = raw16[sl]                                          # [BL, N, Q]
        lt = np.empty((128, 128), bf16)
        lt[0:64] = rc.transpose(0, 2, 1).reshape(64, N)         # rows 8b+q
        lt[64:128] = bf16(-1.0)
        rh = np.zeros((128, 8 * N), bf16)
        rh[0:64] = _IND
        for b in range(BL):
            rh[64 + Q * b:64 + Q * (b + 1), N * b:N * (b + 1)] = \
                rc[b].transpose(1, 0)
        # g row 4b+c = [tl[b, :, c] | br[b, :, c]]
        g = np.empty((32, 2 * N), np.float32)
        g[:, 0:N] = tl[sl].transpose(0, 2, 1).reshape(32, N)
        g[:, N:2 * N] = br[sl].transpose(0, 2, 1).reshape(32, N)
        in_maps.append({"lt": lt, "rh": rh, "g": g})
    return in_maps


def _finish(core_outs):
    pull_total = 0.0
    m_total = 0.0
    for ov, os_ in core_outs:
        ov = np.asarray(ov, dtype=np.float64)
        os_ = np.asarray(os_, dtype=np.float64)
        pull_total += ov[0:32, 0].sum()
        m_total += (os_[:, 0] + os_[:, 1] - ov[:, 1] - ov[:, 2]).sum()
    # per image: 0.5*(sum|A'+2| - sum|A'|) = P_b + N
    pull_all = 0.25 * pull_total / (2 * N)
    push_all = 0.25 * (0.5 * m_total - B * N) / (N * (N - 1))
    return (np.float32(pull_all), np.float32(push_all))


def kernel(pred, target, match):
    from concourse.bass_utils import run_bass_kernel_spmd

    nc = _get_graph()
    in_maps = _make_in_maps(np.asarray(pred), np.asarray(target), np.asarray(match))
    res = run_bass_kernel_spmd(nc, in_maps, core_ids=list(range(M)))
    return _finish([(r["ov"], r["os"]) for r in res.results])


# revision 40
# speedup vs baseline: 1.2096x; 1.0071x over previous
"""Associative-embedding loss kernel for 8 Trainium2 NeuronCores.

Math: per image b, with tl[n,c] = pred[b,c,ty,tx] and br[n,c] = target[b,c,by,bx]
gathered at the N=128 match points:
  pull_b = sum_{n,c} (tl-br)^2 / (2N)
  s'[n]  = sum_c (tl+br),  A'[i,j] = s'[i]-s'[j]   (A = A'/2)
  push_b = (0.5*(sum|A'+2| - sum|A'|) - N) / (N(N-1))
using sum_{ij} relu(1-|A|) = sum|A+1| - sum|A| for antisymmetric A.

Strategy: data-parallel over B (8 images per core). The host shards each
core's 128x2 match points into three small uploads (~320KB/core); HW
indirect DMA is limited to one index per partition per instruction
(~1.3us each, 16 per core), which made an on-device gather the dominant
cost, so the point extraction happens host-side and every loss FLOP runs
on device.

The corner/channel sums that produce s' are folded into the pairwise
matmul contraction (K=128, bf16):
  lhsT rows 8b+q       = raw values v[b, i, q]  (q = 8 corner x channel)
  lhsT rows 64+..      = -1
  rhs rows 8b+q        = 1 on column block b (constant indicator)
  rhs rows 64+8b+q     = v[b, j, q] on column block b, zeros elsewhere
  => out[i, 128b+j] = sum_q v[b,i,q] - sum_q v[b,j,q] = s'_b[i] - s'_b[j]
for all 8 images into one two-bank PSUM tile [128, 1024]. The Scalar
engine accumulates |A'+2| in one pass (Abs with bias via accum_out), the
Vector engine row-reduces |A'| in one pass, and pull comes from an fp32
subtract (GpSimd, 32 partitions) + square-accumulate (DVE) on a separate
[64, 128] upload. rh is split by column halves across two DMA queues so
the first matmul starts half a transfer earlier. bf16 rounding only
perturbs s' by ~0.4%, far inside the 2e-2 gate; pull stays fp32 exact.
Each core returns [128, 8] partial sums folded on the host in fp64.
"""

import numpy as np

B, C, H, W, N = 64, 4, 256, 256, 128
M = 8            # cores
BL = B // M      # images per core
Q = 2 * C        # corner x channel values per point

_GRAPH = None

# constant indicator rows: row 8b+q is 1 on column block b
_IND = np.repeat(np.kron(np.eye(8), np.ones((1, N))), Q, axis=0)


def _build_graph():
    import concourse.bacc as bacc
    import concourse.mybir as mybir
    from concourse.tile import TileContext

    f32 = mybir.dt.float32
    bf16 = mybir.dt.bfloat16
    Alu = mybir.AluOpType
    Act = mybir.ActivationFunctionType
    Axis = mybir.AxisListType

    nc = bacc.Bacc()
    lt_d = nc.declare_dram_parameter("lt", [128, 128], bf16, isOutput=False)
    rh_d = nc.declare_dram_parameter("rh", [128, 8 * N], bf16, isOutput=False)
    g_d = nc.declare_dram_parameter("g", [32, 2 * N], f32, isOutput=False)
    ov_d = nc.declare_dram_parameter("ov", [128, 3], f32, isOutput=True)
    os_d = nc.declare_dram_parameter("os", [128, 2], f32, isOutput=True)

    with TileContext(nc) as tc:
        with (
            tc.tile_pool(name="sb", bufs=1) as pool,
            tc.tile_pool(name="ps", bufs=2, space="PSUM") as psum,
        ):
            # rh (the matmul gate, largest) as one contiguous transfer on the
            # sync queue; lt first on the scalar queue so LDWEIGHTS is never
            # the straggler, then g behind it
            rht = pool.tile([128, 8 * N], bf16)
            nc.sync.dma_start(out=rht[:], in_=rh_d[:])
            ltt = pool.tile([128, 128], bf16)
            nc.scalar.dma_start(out=ltt[:], in_=lt_d[:])
            g = pool.tile([32, 2 * N], f32)
            nc.scalar.dma_start(out=g[:], in_=g_d[:])

            two = pool.tile([128, 1], f32)
            nc.vector.memset(two[:], 2.0)

            # separate result tiles per engine: a shared tile would make the
            # Tile tracker serialize the independent reducers
            accv = pool.tile([128, 3], f32)   # col0 pull (rows 0-31), 1/2 |A'|
            nc.vector.memset(accv[:], 0.0)
            accs = pool.tile([128, 2], f32)   # |A'+2| per bank

            # pull: d = tl - br on GpSimd, square+accumulate on DVE
            dt_ = pool.tile([32, N], f32)
            nc.gpsimd.tensor_sub(dt_[:], g[:, 0:N], g[:, N:2 * N])
            d2 = pool.tile([32, N], f32)
            nc.vector.scalar_tensor_tensor(
                out=d2[:], in0=dt_[:], scalar=0.0, in1=dt_[:],
                op0=Alu.bypass, op1=Alu.mult, accum_out=accv[0:32, 0:1])

            # A'[i, 128b+j] = s'_b[i] - s'_b[j], 4 images per PSUM bank;
            # separate bank tiles so each bank's reducers start right after
            # its matmul instead of after both
            bankA = psum.tile([128, 512], f32, name="bankA", tag="a")
            bankB = psum.tile([128, 512], f32, name="bankB", tag="b")
            nc.tensor.matmul(out=bankA[:], lhsT=ltt[:], rhs=rht[:, 0:512],
                             start=True, stop=True)
            nc.tensor.matmul(out=bankB[:], lhsT=ltt[:], rhs=rht[:, 512:1024],
                             start=True, stop=True)

            # accs = rowsum |A'+2| per bank; accv col1/2 = rowsum |A'|
            scr = pool.tile([128, 8 * N], f32)
            nc.scalar.activation(
                out=scr[:, 0:512], in_=bankA[:], func=Act.Abs,
                bias=two[:, 0:1], scale=1.0, accum_out=accs[:, 0:1])
            nc.scalar.activation(
                out=scr[:, 512:1024], in_=bankB[:], func=Act.Abs,
                bias=two[:, 0:1], scale=1.0, accum_out=accs[:, 1:2])
            nc.vector.tensor_reduce(
                out=accv[:, 1:2], in_=bankA[:], axis=Axis.X,
                op=Alu.add, apply_absolute_value=True)
            nc.vector.tensor_reduce(
                out=accv[:, 2:3], in_=bankB[:], axis=Axis.X,
                op=Alu.add, apply_absolute_value=True)

            nc.sync.dma_start(out=ov_d[:], in_=accv[:])
            nc.scalar.dma_start(out=os_d[:], in_=accs[:])
    nc.finalize()
    return nc


def _get_graph():
    global _GRAPH
    if _GRAPH is None:
        _GRAPH = _build_graph()
    return _GRAPH


def _make_in_maps(pred, target, match):
    import ml_dtypes

    bf16 = ml_dtypes.bfloat16
    barr = np.arange(B)[:, None]
    tl = pred[barr, :, match[:, :, 0, 0], match[:, :, 0, 1]]    # [B, N, C]
    br = target[barr, :, match[:, :, 1, 0], match[:, :, 1, 1]]  # [B, N, C]
    raw = np.concatenate([tl, br], axis=-1)                     # [B, N, Q]
    raw16 = raw.astype(bf16)

    in_maps = []
    for i in range(M):
        sl = slice(i * BL, (i + 1) * BL)
        rc = raw16[sl]                                          # [BL, N, Q]
        lt = np.empty((128, 128), bf16)
        lt[0:64] = rc.transpose(0, 2, 1).reshape(64, N)         # rows 8b+q
        lt[64:128] = bf16(-1.0)
        rh = np.zeros((128, 8 * N), bf16)
        rh[0:64] = _IND
        for b in range(BL):
            rh[64 + Q * b:64 + Q * (b + 1), N * b:N * (b + 1)] = \
                rc[b].transpose(1, 0)
        # g row 4b+c = [tl[b, :, c] | br[b, :, c]]
        g = np.empty((32, 2 * N), np.float32)
        g[:, 0:N] = tl[sl].transpose(0, 2, 1).reshape(32, N)
        g[:, N:2 * N] = br[sl].transpose(0, 2, 1).reshape(32, N)
        in_maps.append({"lt": lt, "rh": rh, "g": g})
    return in_maps


def _finish(core_outs):
    pull_total = 0.0
    m_total = 0.0
    for ov, os_ in core_outs:
        ov = np.asarray(ov, dtype=np.float64)
        os_ = np.asarray(os_, dtype=np.float64)
        pull_total += ov[0:32, 0].sum()
        m_total += (os_[:, 0] + os_[:, 1] - ov[:, 1] - ov[:, 2]).sum()
    # per image: 0.5*(sum|A'+2| - sum|A'|) = P_b + N
    pull_all = 0.25 * pull_total / (2 * N)
    push_all = 0.25 * (0.5 * m_total - B * N) / (N * (N - 1))
    return (np.float32(pull_all), np.float32(push_all))


def kernel(pred, target, match):
    from concourse.bass_utils import run_bass_kernel_spmd

    nc = _get_graph()
    in_maps = _make_in_maps(np.asarray(pred), np.asarray(target), np.asarray(match))
    res = run_bass_kernel_spmd(nc, in_maps, core_ids=list(range(M)))
    return _finish([(r["ov"], r["os"]) for r in res.results])


# revision 46
# speedup vs baseline: 1.4736x; 1.2183x over previous
"""Associative-embedding loss kernel for 8 Trainium2 NeuronCores.

Math: per image b, with tl[n,c] = pred[b,c,ty,tx] and br[n,c] = target[b,c,by,bx]
gathered at the N=128 match points:
  pull_b = sum_{n,c} (tl-br)^2 / (2N)
  s'[n]  = sum_c (tl+br),  A'[i,j] = s'[i]-s'[j]   (A = A'/2)
  push_b = (0.5*(sum|A'+2| - sum|A'|) - N) / (N(N-1))
using sum_{ij} relu(1-|A|) = sum|A+1| - sum|A| for antisymmetric A.

Strategy: data-parallel over B (8 images per core). The host shards each
core's 128x2 match points into three small uploads (~320KB/core); HW
indirect DMA is limited to one index per partition per instruction
(~1.3us each, 16 per core), which made an on-device gather the dominant
cost, so the point extraction happens host-side and every loss FLOP runs
on device.

The corner/channel sums that produce s' are folded into the pairwise
matmul contraction (K=128, bf16):
  lhsT rows 8b+q       = raw values v[b, i, q]  (q = 8 corner x channel)
  lhsT rows 64+..      = -1
  rhs rows 8b+q        = 1 on column block b (constant indicator)
  rhs rows 64+8b+q     = v[b, j, q] on column block b, zeros elsewhere
  => out[i, 128b+j] = sum_q v[b,i,q] - sum_q v[b,j,q] = s'_b[i] - s'_b[j]
for all 8 images into one two-bank PSUM tile [128, 1024]. The Scalar
engine accumulates |A'+2| in one pass (Abs with bias via accum_out), the
Vector engine row-reduces |A'| in one pass, and pull comes from an fp32
subtract (GpSimd, 32 partitions) + square-accumulate (DVE) on a separate
[64, 128] upload. rh is split by column halves across two DMA queues so
the first matmul starts half a transfer earlier. bf16 rounding only
perturbs s' by ~0.4%, far inside the 2e-2 gate; pull stays fp32 exact.
Each core returns [128, 8] partial sums folded on the host in fp64.
"""

import numpy as np

B, C, H, W, N = 64, 4, 256, 256, 128
M = 8            # cores
BL = B // M      # images per core
Q = 2 * C        # corner x channel values per point

_GRAPH = None

# constant indicator rows: row 8b+q is 1 on column block b
_IND = np.repeat(np.kron(np.eye(8), np.ones((1, N))), Q, axis=0)
_IDENT = np.eye(128, dtype=np.float32)


def _build_graph():
    import concourse.bacc as bacc
    import concourse.mybir as mybir
    from concourse.tile import TileContext

    f32 = mybir.dt.float32
    bf16 = mybir.dt.bfloat16
    Alu = mybir.AluOpType
    Act = mybir.ActivationFunctionType
    Axis = mybir.AxisListType

    nc = bacc.Bacc()
    lt_d = nc.declare_dram_parameter("lt", [128, 128], bf16, isOutput=False)
    rh_d = nc.declare_dram_parameter("rh", [128, 8 * N], bf16, isOutput=False)
    g_d = nc.declare_dram_parameter("g", [32, 2 * N], f32, isOutput=False)
    id_d = nc.declare_dram_parameter("ident", [128, 128], f32, isOutput=False)
    ov_d = nc.declare_dram_parameter("ov", [3, 128], f32, isOutput=True)
    os_d = nc.declare_dram_parameter("os", [2, 128], f32, isOutput=True)

    with TileContext(nc) as tc:
        with (
            tc.tile_pool(name="sb", bufs=1) as pool,
            tc.tile_pool(name="ps", bufs=2, space="PSUM") as psum,
        ):
            # rh (the matmul gate, largest) as one contiguous transfer on the
            # sync queue; lt first on the scalar queue so LDWEIGHTS is never
            # the straggler, then g behind it
            rht = pool.tile([128, 8 * N], bf16)
            nc.sync.dma_start(out=rht[:], in_=rh_d[:])
            ltt = pool.tile([128, 128], bf16)
            nc.scalar.dma_start(out=ltt[:], in_=lt_d[:])
            g = pool.tile([32, 2 * N], f32)
            nc.scalar.dma_start(out=g[:], in_=g_d[:])
            ident = pool.tile([128, 128], f32)
            nc.scalar.dma_start(out=ident[:], in_=id_d[:])

            two = pool.tile([128, 1], f32)
            nc.vector.memset(two[:], 2.0)

            # separate result tiles per engine: a shared tile would make the
            # Tile tracker serialize the independent reducers
            accv = pool.tile([128, 3], f32)   # col0 pull (rows 0-31), 1/2 |A'|
            nc.vector.memset(accv[:], 0.0)
            accs = pool.tile([128, 2], f32)   # |A'+2| per bank

            # pull: d = tl - br on GpSimd, square+accumulate on DVE
            dt_ = pool.tile([32, N], f32)
            nc.gpsimd.tensor_sub(dt_[:], g[:, 0:N], g[:, N:2 * N])
            d2 = pool.tile([32, N], f32)
            nc.vector.scalar_tensor_tensor(
                out=d2[:], in0=dt_[:], scalar=0.0, in1=dt_[:],
                op0=Alu.bypass, op1=Alu.mult, accum_out=accv[0:32, 0:1])

            # A'[i, 128b+j] = s'_b[i] - s'_b[j], 4 images per PSUM bank;
            # separate bank tiles so each bank's reducers start right after
            # its matmul instead of after both
            bankA = psum.tile([128, 512], f32, name="bankA", tag="a")
            bankB = psum.tile([128, 512], f32, name="bankB", tag="b")
            nc.tensor.matmul(out=bankA[:], lhsT=ltt[:], rhs=rht[:, 0:512],
                             start=True, stop=True)
            nc.tensor.matmul(out=bankB[:], lhsT=ltt[:], rhs=rht[:, 512:1024],
                             start=True, stop=True)

            # accs = rowsum |A'+2| per bank; accv col1/2 = rowsum |A'|.
            # Ping-pong bank order (Scalar: B,A; DVE: A,B) so the engines
            # never contend for the same PSUM bank's read port.
            scr = pool.tile([128, 8 * N], f32)
            nc.scalar.activation(
                out=scr[:, 512:1024], in_=bankB[:], func=Act.Abs,
                bias=two[:, 0:1], scale=1.0, accum_out=accs[:, 1:2])
            nc.scalar.activation(
                out=scr[:, 0:512], in_=bankA[:], func=Act.Abs,
                bias=two[:, 0:1], scale=1.0, accum_out=accs[:, 0:1])
            nc.vector.tensor_reduce(
                out=accv[:, 1:2], in_=bankA[:], axis=Axis.X,
                op=Alu.add, apply_absolute_value=True)
            nc.vector.tensor_reduce(
                out=accv[:, 2:3], in_=bankB[:], axis=Axis.X,
                op=Alu.add, apply_absolute_value=True)

            # a [128, few] result DMA is 128 tiny descriptors whose HBM write
            # completion dominates the tail; transpose on the idle PE first
            # so each output is a handful of 512B descriptors instead
            tv_ps = psum.tile([3, 128], f32, name="tv", tag="tv")
            ts_ps = psum.tile([2, 128], f32, name="ts", tag="ts")
            nc.tensor.transpose(out=tv_ps[:], in_=accv[:], identity=ident[:])
            nc.tensor.transpose(out=ts_ps[:], in_=accs[:], identity=ident[:])
            tvs = pool.tile([3, 128], f32)
            nc.vector.tensor_copy(tvs[:], tv_ps[:])
            tss = pool.tile([2, 128], f32)
            nc.scalar.copy(tss[:], ts_ps[:])

            nc.sync.dma_start(out=ov_d[:], in_=tvs[:])
            nc.scalar.dma_start(out=os_d[:], in_=tss[:])
    nc.finalize()
    return nc


def _get_graph():
    global _GRAPH
    if _GRAPH is None:
        _GRAPH = _build_graph()
    return _GRAPH


def _make_in_maps(pred, target, match):
    import ml_dtypes

    bf16 = ml_dtypes.bfloat16
    barr = np.arange(B)[:, None]
    tl = pred[barr, :, match[:, :, 0, 0], match[:, :, 0, 1]]    # [B, N, C]
    br = target[barr, :, match[:, :, 1, 0], match[:, :, 1, 1]]  # [B, N, C]
    raw = np.concatenate([tl, br], axis=-1)                     # [B, N, Q]
    raw16 = raw.astype(bf16)

    in_maps = []
    for i in range(M):
        sl = slice(i * BL, (i + 1) * BL)
        rc = raw16[sl]                                          # [BL, N, Q]
        lt = np.empty((128, 128), bf16)
        lt[0:64] = rc.transpose(0, 2, 1).reshape(64, N)         # rows 8b+q
        lt[64:128] = bf16(-1.0)
        rh = np.zeros((128, 8 * N), bf16)
        rh[0:64] = _IND
        for b in range(BL):
            rh[64 + Q * b:64 + Q * (b + 1), N * b:N * (b + 1)] = \
                rc[b].transpose(1, 0)
        # g row 4b+c = [tl[b, :, c] | br[b, :, c]]
        g = np.empty((32, 2 * N), np.float32)
        g[:, 0:N] = tl[sl].transpose(0, 2, 1).reshape(32, N)
        g[:, N:2 * N] = br[sl].transpose(0, 2, 1).reshape(32, N)
        in_maps.append({"lt": lt, "rh": rh, "g": g, "ident": _IDENT})
    return in_maps


def _finish(core_outs):
    pull_total = 0.0
    m_total = 0.0
    for ov, os_ in core_outs:
        ov = np.asarray(ov, dtype=np.float64)
        os_ = np.asarray(os_, dtype=np.float64)
        pull_total += ov[0, 0:32].sum()
        m_total += (os_[0, :] + os_[1, :] - ov[1, :] - ov[2, :]).sum()
    # per image: 0.5*(sum|A'+2| - sum|A'|) = P_b + N
    pull_all = 0.25 * pull_total / (2 * N)
    push_all = 0.25 * (0.5 * m_total - B * N) / (N * (N - 1))
    return (np.float32(pull_all), np.float32(push_all))


def kernel(pred, target, match):
    from concourse.bass_utils import run_bass_kernel_spmd

    nc = _get_graph()
    in_maps = _make_in_maps(np.asarray(pred), np.asarray(target), np.asarray(match))
    res = run_bass_kernel_spmd(nc, in_maps, core_ids=list(range(M)))
    return _finish([(r["ov"], r["os"]) for r in res.results])
